# revision 22
# baseline (speedup 1.0000x reference)
"""Trainium2 Bass kernel: monomials x^a y^b z^c (a+b+c <= 3) for N=2M points.

Data-parallel across 8 NeuronCores; each core gets N/8 = 250k points padded
to 128*1960. The trivial columns (1, x, y, z) are assembled host-side; the
device computes the 16 degree>=2 monomials in bf16 (one truncation vs the
f32 reference, ~8e-3 max rel err, well under the 2e-2 gate) to halve the
HBM write traffic.

Layout is PLANAR to keep every engine access unit-stride with long runs
(strided APs with short inner counts run at 2-5 cyc/elem on DVE/ACT):
  host in  : per tile [128, 3, f]  (x-plane, y-plane, z-plane per partition)
  SBUF it  : [P, 3f] f32           x = [0:f], y = [f:2f], z = [2f:3f]
  SBUF ot  : [P, 16f] bf16         monomial k = [kf:(k+1)f]
  host out : per tile [128, 16, f] -> transposed to [points, 16] on host
Device monomials: 0:x2 1:xy 2:xz 3:y2 4:yz 5:z2
             6:x3 7:x2y 8:x2z 9:xy2 10:xyz 11:xz2 12:y3 13:y2z 14:yz2 15:z3
ACT: squares -> planes 0,3,5. DVE: products (in0 broadcast along a step-0
middle dim): xy|xz <- x*(y,z); yz; deg3 = x*(planes0:6)->6:12,
y*(planes3:6)->12:15, z*plane5->15. SP: all DMAs + out triggers (keeps
DVE/ACT from blocking on each other's DMA waits).

Tiles have VARIABLE sizes (F_LIST): a small first tile fills the pipeline
fast (ramp) and a small last tile shortens the store tail. All tiles are
SBUF-resident (no slot reuse -> no WAR waits). Out-DMAs are split into
planes 0:12 (ready after the 6-wide deg3 op) and 12:16.

Raw bass (no Tile): this walrus rejects >1 sync-wait per instruction, so
all waits are standalone wait_ge ops; DMA sems are per-tile so at most one
DMA in flight per sem keeps wait values unambiguous.
"""

import sys
from contextlib import ExitStack

if "/opt/trn_rl_repo" not in sys.path:
    sys.path.insert(0, "/opt/trn_rl_repo")

import numpy as np
import concourse.bass as bass
import concourse.mybir as mybir
from concourse.bass_utils import run_bass_kernel_spmd

P = 128
K = 20
KD = 16  # device-computed columns (degree >= 2)
N_TOTAL = 2_000_000
N_CORES = 8
N_CORE = N_TOTAL // N_CORES  # 250_000
F_TOTAL = 1960
F_LIST = [96, 584, 640, 544, 96]  # sums to F_TOTAL; 8-divisible for aligned bf16 runs
N_PAD = P * F_TOTAL  # 250_880

AF = mybir.ActivationFunctionType
F32 = mybir.dt.float32
BF16 = mybir.dt.bfloat16


def build(nc: bass.Bass, f_list) -> bass.Bass:
    t_total = len(f_list)
    f_sum = sum(f_list)
    offs = np.concatenate([[0], np.cumsum(f_list)]).astype(int)  # per-partition

    v = nc.declare_dram_parameter("vectors", [P * 3 * f_sum], F32, isOutput=False)
    o = nc.declare_dram_parameter("out", [P * KD * f_sum], BF16, isOutput=True)

    with ExitStack() as ctx:
        itb = ctx.enter_context(nc.sbuf_tensor("itb", [P, 3 * f_sum], F32))
        itb16 = ctx.enter_context(nc.sbuf_tensor("itb16", [P, 3 * f_sum], BF16))
        otb = ctx.enter_context(nc.sbuf_tensor("otb", [P, KD * f_sum], BF16))
        s_in = [ctx.enter_context(nc.semaphore(f"s_in{i}")) for i in range(t_total)]
        s_out = [ctx.enter_context(nc.semaphore(f"s_out{i}")) for i in range(t_total)]
        s_v = ctx.enter_context(nc.semaphore("s_v"))
        s_va = ctx.enter_context(nc.semaphore("s_va"))
        s_d = ctx.enter_context(nc.semaphore("s_d"))
        s_q = ctx.enter_context(nc.semaphore("s_q"))
        block = ctx.enter_context(nc.Block(no_gpsimd_drain=True))

        def it_flat(t):
            return itb.ap()[:, 3 * offs[t] : 3 * offs[t + 1]]

        def it16_flat(t):
            return itb16.ap()[:, 3 * offs[t] : 3 * offs[t + 1]]

        def ot_flat(t):
            return otb.ap()[:, KD * offs[t] : KD * offs[t + 1]]

        def v_dram(t):
            return v[P * 3 * offs[t] : P * 3 * offs[t + 1]].rearrange(
                "(p q) -> p q", p=P
            )

        def plane(base, t, k, w=1):
            """[P, w, f_t] view of planes k..k+w of a per-tile flat AP."""
            f = f_list[t]
            return base[:, k * f : (k + w) * f].rearrange("p (c f) -> p c f", f=f)

        def bcast(base, t, k, w):
            f = f_list[t]
            return plane(base, t, k, 1).broadcast_to([P, w, f])

        @block.sync
        def _(sync):
            # Front-load all input DMAs (ins finish before outs need the
            # SDMA engines), then trigger out-DMAs from here: SP is
            # otherwise idle, so DVE/ACT never block on a DMA trigger
            # waiting for the other engine.
            for t in range(t_total):
                sync.dma_start(out=it_flat(t), in_=v_dram(t)).then_inc(s_in[t], 16)
            for t in range(t_total):
                f = f_list[t]
                base = P * KD * offs[t]
                od = o[base : base + P * KD * f].rearrange("(p q) -> p q", p=P)
                # Planes 0:12 are done after the 6-wide deg3 op (s_va);
                # 12:16 after the tile completes (s_v). Splitting starts
                # the store earlier and shortens the tail.
                sync.wait_ge(s_va, t + 1)
                sync.dma_start(
                    out=od[:, 0 : 12 * f], in_=ot_flat(t)[:, 0 : 12 * f]
                ).then_inc(s_out[t], 16)
                sync.wait_ge(s_v, t + 1)
                sync.dma_start(
                    out=od[:, 12 * f : KD * f], in_=ot_flat(t)[:, 12 * f : KD * f]
                ).then_inc(s_out[t], 16)
            for t in range(t_total):
                sync.wait_ge(s_out[t], 32)

        @block.vector
        def _(vector):
            for t in range(t_total):
                it = it_flat(t)
                ot = ot_flat(t)
                vector.wait_ge(s_in[t], 16)
                # xy, xz -> planes 1:3
                nc.vector.tensor_mul(
                    plane(ot, t, 1, 2), bcast(it, t, 0, 2), plane(it, t, 1, 2)
                )
                # yz -> plane 4
                nc.vector.tensor_mul(
                    plane(ot, t, 4), plane(it, t, 1), plane(it, t, 2)
                ).then_inc(s_d, 1)
                # deg3: both operands bf16 (ACT-made input copies) to
                # engage the DVE 2x perf mode; reads ACT's squares/copies
                # (s_q) and our own xy/xz/yz through the pipe (s_d).
                it16 = it16_flat(t)
                vector.wait_ge(s_d, t + 1)
                vector.wait_ge(s_q, t + 1)
                nc.vector.tensor_mul(
                    plane(ot, t, 6, 6), bcast(it16, t, 0, 6), plane(ot, t, 0, 6)
                ).then_inc(s_va, 1)
                nc.vector.tensor_mul(
                    plane(ot, t, 12, 3), bcast(it16, t, 1, 3), plane(ot, t, 3, 3)
                )
                nc.vector.tensor_mul(
                    plane(ot, t, 15), plane(it16, t, 2), plane(ot, t, 5)
                ).then_inc(s_v, 1)

        @block.scalar
        def _(scalar):
            for t in range(t_total):
                it = it_flat(t)
                ot = ot_flat(t)
                it16 = it16_flat(t)
                scalar.wait_ge(s_in[t], 16)
                nc.scalar.square(plane(ot, t, 0), plane(it, t, 0))
                nc.scalar.square(plane(ot, t, 3), plane(it, t, 1))
                nc.scalar.square(plane(ot, t, 5), plane(it, t, 2))
                # f32 -> bf16 input copies; in-order completion implies
                # the squares above are also done when this inc fires.
                nc.scalar.copy(it16, it).then_inc(s_q, 1)

    return nc


_CACHE: dict[str, object] = {}


def _get_nc() -> bass.Bass:
    if "nc" not in _CACHE:
        nc = bass.Bass()
        build(nc, F_LIST)
        _CACHE["nc"] = nc
    return _CACHE["nc"]  # type: ignore[return-value]


def run_spmd(in_maps, trace=False, **kw):
    return run_bass_kernel_spmd(
        _get_nc(), in_maps, core_ids=list(range(N_CORES)), trace=trace, **kw
    )


def to_planar(shard: np.ndarray, f_list=F_LIST) -> np.ndarray:
    """[n_pad, 3] f32 -> flat [P*3*sum(f)] planar per-tile blocks."""
    parts = []
    pos = 0
    for f in f_list:
        blk = shard[pos : pos + P * f].reshape(P, f, 3).transpose(0, 2, 1)
        parts.append(blk.reshape(-1))
        pos += P * f
    return np.ascontiguousarray(np.concatenate(parts))


def from_planar(dev_out: np.ndarray, f_list=F_LIST) -> np.ndarray:
    """flat [P*16*sum(f)] (any dtype) -> [n_pad, 16] f32."""
    arr = np.asarray(dev_out, dtype=np.float32).reshape(-1)
    outs = []
    pos = 0
    for f in f_list:
        blk = arr[pos : pos + P * KD * f].reshape(P, KD, f).transpose(0, 2, 1)
        outs.append(blk.reshape(P * f, KD))
        pos += P * KD * f
    return np.concatenate(outs)


def make_in_maps(vectors: np.ndarray):
    vectors = np.ascontiguousarray(np.asarray(vectors, dtype=np.float32))
    assert vectors.shape == (N_TOTAL, 3)
    shards = vectors.reshape(N_CORES, N_CORE, 3)
    in_maps = []
    for i in range(N_CORES):
        buf = np.zeros((N_PAD, 3), dtype=np.float32)
        buf[:N_CORE] = shards[i]
        in_maps.append({"vectors": to_planar(buf)})
    return in_maps


def kernel(vectors: np.ndarray) -> np.ndarray:
    vec32 = np.ascontiguousarray(np.asarray(vectors, dtype=np.float32))
    res = run_spmd(make_in_maps(vec32))
    out = np.empty((N_TOTAL, K), dtype=np.float32)
    out[:, 0] = 1.0
    out[:, 1:4] = vec32  # degree-1 monomials are the input, exactly
    for i in range(N_CORES):
        out[i * N_CORE : (i + 1) * N_CORE, 4:] = from_planar(res.results[i]["out"])[
            :N_CORE
        ]
    return out


# revision 23
# speedup vs baseline: 1.1106x; 1.1106x over previous
"""Trainium2 Bass kernel: monomials x^a y^b z^c (a+b+c <= 3) for N=2M points.

Data-parallel across 8 NeuronCores; each core gets N/8 = 250k points padded
to 128*1960. The trivial columns (1, x, y, z) are assembled host-side; the
device computes the 16 degree>=2 monomials in bf16 to halve the HBM write
traffic (measured 1.07e-2 max rel err / 2.6e-3 l2 vs the f32 reference --
bf16 round-to-nearest is +-2^-8 per store, three compounded stores for
degree 3 -- comfortably under the 2e-2 gate).

Layout is PLANAR to keep every engine access unit-stride with long runs
(strided APs with short inner counts run at 2-5 cyc/elem on DVE/ACT):
  host in  : per tile [128, 3, f]  (x-plane, y-plane, z-plane per partition)
  SBUF it  : [P, 3f] f32           x = [0:f], y = [f:2f], z = [2f:3f]
  SBUF ot  : [P, 16f] bf16         monomial k = [kf:(k+1)f]
  host out : per tile [128, 16, f] -> transposed to [points, 16] on host
Device monomials: 0:x2 1:xy 2:xz 3:y2 4:yz 5:z2
             6:x3 7:x2y 8:x2z 9:xy2 10:xyz 11:xz2 12:y3 13:y2z 14:yz2 15:z3
ACT: squares -> planes 0,3,5 plus bf16 copies of x,y,z. DVE: cross
products from f32 inputs at 1x; the 10F degree-3 block multiplies two bf16
operands so the DVE 2x_1P perf mode engages (2 elem/cycle). SP: all DMAs +
out triggers (keeps DVE/ACT from blocking on each other's DMA waits).

Tiles have VARIABLE sizes (F_LIST): a small first tile fills the pipeline
fast (ramp) and a small last tile shortens the store tail. All tiles are
SBUF-resident (no slot reuse -> no WAR waits). Out-DMAs are split into
planes 0:12 (ready after the 6-wide deg3 op) and 12:16.

Raw bass (no Tile): this walrus rejects >1 sync-wait per instruction, so
all waits are standalone wait_ge ops; DMA sems are per-tile so at most one
DMA in flight per sem keeps wait values unambiguous.
"""

import sys
from contextlib import ExitStack

if "/opt/trn_rl_repo" not in sys.path:
    sys.path.insert(0, "/opt/trn_rl_repo")

import numpy as np
import concourse.bass as bass
import concourse.mybir as mybir
from concourse.bass_utils import run_bass_kernel_spmd

P = 128
K = 20
KD = 16  # device-computed columns (degree >= 2)
N_TOTAL = 2_000_000
N_CORES = 8
N_CORE = N_TOTAL // N_CORES  # 250_000
F_TOTAL = 1960
F_LIST = [96, 584, 640, 544, 96]  # sums to F_TOTAL; 8-divisible for aligned bf16 runs
N_PAD = P * F_TOTAL  # 250_880

AF = mybir.ActivationFunctionType
F32 = mybir.dt.float32
BF16 = mybir.dt.bfloat16


def build(nc: bass.Bass, f_list) -> bass.Bass:
    t_total = len(f_list)
    f_sum = sum(f_list)
    offs = np.concatenate([[0], np.cumsum(f_list)]).astype(int)  # per-partition

    v = nc.declare_dram_parameter("vectors", [P * 3 * f_sum], F32, isOutput=False)
    o = nc.declare_dram_parameter("out", [P * KD * f_sum], BF16, isOutput=True)

    with ExitStack() as ctx:
        itb = ctx.enter_context(nc.sbuf_tensor("itb", [P, 3 * f_sum], F32))
        itb16 = ctx.enter_context(nc.sbuf_tensor("itb16", [P, 3 * f_sum], BF16))
        otb = ctx.enter_context(nc.sbuf_tensor("otb", [P, KD * f_sum], BF16))
        s_in = [ctx.enter_context(nc.semaphore(f"s_in{i}")) for i in range(t_total)]
        s_out = [ctx.enter_context(nc.semaphore(f"s_out{i}")) for i in range(t_total)]
        s_v = ctx.enter_context(nc.semaphore("s_v"))
        s_va = ctx.enter_context(nc.semaphore("s_va"))
        s_d = ctx.enter_context(nc.semaphore("s_d"))
        s_q = ctx.enter_context(nc.semaphore("s_q"))
        block = ctx.enter_context(nc.Block(no_gpsimd_drain=True))

        def it_flat(t):
            return itb.ap()[:, 3 * offs[t] : 3 * offs[t + 1]]

        def it16_flat(t):
            return itb16.ap()[:, 3 * offs[t] : 3 * offs[t + 1]]

        def ot_flat(t):
            return otb.ap()[:, KD * offs[t] : KD * offs[t + 1]]

        def v_dram(t):
            return v[P * 3 * offs[t] : P * 3 * offs[t + 1]].rearrange(
                "(p q) -> p q", p=P
            )

        def plane(base, t, k, w=1):
            """[P, w, f_t] view of planes k..k+w of a per-tile flat AP."""
            f = f_list[t]
            return base[:, k * f : (k + w) * f].rearrange("p (c f) -> p c f", f=f)

        def bcast(base, t, k, w):
            f = f_list[t]
            return plane(base, t, k, 1).broadcast_to([P, w, f])

        @block.sync
        def _(sync):
            # Front-load all input DMAs (ins finish before outs need the
            # SDMA engines), then trigger out-DMAs from here: SP is
            # otherwise idle, so DVE/ACT never block on a DMA trigger
            # waiting for the other engine.
            for t in range(t_total):
                sync.dma_start(out=it_flat(t), in_=v_dram(t)).then_inc(s_in[t], 16)
            for t in range(t_total):
                f = f_list[t]
                base = P * KD * offs[t]
                od = o[base : base + P * KD * f].rearrange("(p q) -> p q", p=P)
                # Planes 0:12 are done after the 6-wide deg3 op (s_va);
                # 12:16 after the tile completes (s_v). Splitting starts
                # the store earlier and shortens the tail.
                sync.wait_ge(s_va, t + 1)
                sync.dma_start(
                    out=od[:, 0 : 12 * f], in_=ot_flat(t)[:, 0 : 12 * f]
                ).then_inc(s_out[t], 16)
                sync.wait_ge(s_v, t + 1)
                sync.dma_start(
                    out=od[:, 12 * f : KD * f], in_=ot_flat(t)[:, 12 * f : KD * f]
                ).then_inc(s_out[t], 16)
            for t in range(t_total):
                sync.wait_ge(s_out[t], 32)

        @block.vector
        def _(vector):
            for t in range(t_total):
                it = it_flat(t)
                ot = ot_flat(t)
                vector.wait_ge(s_in[t], 16)
                # xy, xz -> planes 1:3
                nc.vector.tensor_mul(
                    plane(ot, t, 1, 2), bcast(it, t, 0, 2), plane(it, t, 1, 2)
                )
                # yz -> plane 4
                nc.vector.tensor_mul(
                    plane(ot, t, 4), plane(it, t, 1), plane(it, t, 2)
                ).then_inc(s_d, 1)
                # deg3: both operands bf16 (ACT-made input copies) to
                # engage the DVE 2x perf mode; reads ACT's squares/copies
                # (s_q) and our own xy/xz/yz through the pipe (s_d).
                it16 = it16_flat(t)
                vector.wait_ge(s_d, t + 1)
                vector.wait_ge(s_q, t + 1)
                nc.vector.tensor_mul(
                    plane(ot, t, 6, 6), bcast(it16, t, 0, 6), plane(ot, t, 0, 6)
                ).then_inc(s_va, 1)
                nc.vector.tensor_mul(
                    plane(ot, t, 12, 3), bcast(it16, t, 1, 3), plane(ot, t, 3, 3)
                )
                nc.vector.tensor_mul(
                    plane(ot, t, 15), plane(it16, t, 2), plane(ot, t, 5)
                ).then_inc(s_v, 1)

        @block.scalar
        def _(scalar):
            for t in range(t_total):
                it = it_flat(t)
                ot = ot_flat(t)
                it16 = it16_flat(t)
                scalar.wait_ge(s_in[t], 16)
                nc.scalar.square(plane(ot, t, 0), plane(it, t, 0))
                nc.scalar.square(plane(ot, t, 3), plane(it, t, 1))
                nc.scalar.square(plane(ot, t, 5), plane(it, t, 2))
                # f32 -> bf16 input copies; in-order completion implies
                # the squares above are also done when this inc fires.
                nc.scalar.copy(it16, it).then_inc(s_q, 1)

    return nc


_CACHE: dict[str, object] = {}


def _get_nc() -> bass.Bass:
    if "nc" not in _CACHE:
        nc = bass.Bass()
        build(nc, F_LIST)
        _CACHE["nc"] = nc
    return _CACHE["nc"]  # type: ignore[return-value]


def run_spmd(in_maps, trace=False, **kw):
    return run_bass_kernel_spmd(
        _get_nc(), in_maps, core_ids=list(range(N_CORES)), trace=trace, **kw
    )


def to_planar(shard: np.ndarray, f_list=F_LIST) -> np.ndarray:
    """[n_pad, 3] f32 -> flat [P*3*sum(f)] planar per-tile blocks."""
    parts = []
    pos = 0
    for f in f_list:
        blk = shard[pos : pos + P * f].reshape(P, f, 3).transpose(0, 2, 1)
        parts.append(blk.reshape(-1))
        pos += P * f
    return np.ascontiguousarray(np.concatenate(parts))


def from_planar(dev_out: np.ndarray, f_list=F_LIST) -> np.ndarray:
    """flat [P*16*sum(f)] (any dtype) -> [n_pad, 16] f32."""
    arr = np.asarray(dev_out, dtype=np.float32).reshape(-1)
    outs = []
    pos = 0
    for f in f_list:
        blk = arr[pos : pos + P * KD * f].reshape(P, KD, f).transpose(0, 2, 1)
        outs.append(blk.reshape(P * f, KD))
        pos += P * KD * f
    return np.concatenate(outs)


def make_in_maps(vectors: np.ndarray):
    vectors = np.ascontiguousarray(np.asarray(vectors, dtype=np.float32))
    assert vectors.shape == (N_TOTAL, 3)
    shards = vectors.reshape(N_CORES, N_CORE, 3)
    in_maps = []
    for i in range(N_CORES):
        buf = np.zeros((N_PAD, 3), dtype=np.float32)
        buf[:N_CORE] = shards[i]
        in_maps.append({"vectors": to_planar(buf)})
    return in_maps


def kernel(vectors: np.ndarray) -> np.ndarray:
    vec32 = np.ascontiguousarray(np.asarray(vectors, dtype=np.float32))
    res = run_spmd(make_in_maps(vec32))
    out = np.empty((N_TOTAL, K), dtype=np.float32)
    out[:, 0] = 1.0
    out[:, 1:4] = vec32  # degree-1 monomials are the input, exactly
    for i in range(N_CORES):
        out[i * N_CORE : (i + 1) * N_CORE, 4:] = from_planar(res.results[i]["out"])[
            :N_CORE
        ]
    return out
